# revision 26
# baseline (speedup 1.0000x reference)
"""3-layer GATv2 (heads=1, eval) on 8 Trainium2 NeuronCores — Bass/Tile.

kernel(**inputs) takes the FULL inputs (x [100000,128] f32, Wl/Wr [3,128,128],
att [3,128], b [3,128], edge_index [2,1600000] int64) and returns the FULL
[100000, 128] float32 output.

Strategy (graph/data parallel; node-partitioned by dst):
  * core c owns dst nodes [c*12500, (c+1)*12500). Edges grouped on the host
    by (dst block of 128 nodes, src bucket) with per-bucket slot budgets
    B1K[k] (multiples of 128, max over cores/blocks); pad slots use idx 0
    and a dloc sentinel (512) whose one-hot row is all-zero downstream.
  * per layer, XL = h@Wl over all N rows (bf16, AllGather'd in 4 non-uniform
    chunks [4095,4095,3310,1000] local rows so the final chunk's gather
    barrier at the next layer boundary is small) and local XR = h@Wr live
    in DRAM. XLf is a Shared-scratchpad tensor (fast HBM-HBM AllGather).
    Only xl[src] is fetched per edge, with SWDGE dma_gather per
    (block-pair, bucket), round-robin over 4 SWDGE queues. xr[dst] is
    expanded on-chip from the contiguous 128-row XR block via PE one-hot
    matmuls.
  * per block: one-hot O[slot,node] built in ONE DVE op (iota==dloc with
    stride-0 broadcast); flat 4-group chunks: O_T = PE-transpose(O_g), one
    ACT copy per chunk, v = O_T_g.T@XR_blk + wide identity matmuls over the
    gathered xl (<=512-col runs per bucket) accumulating in PSUM;
    z = Prelu(v, 0.2) per chunk via ACT; t = z*att (DVE); e = sum_d t via
    halve-add + reduce (DVE); w = exp(e) (ACT, no segment-max: |e| small
    for this model); Y' = [w*xl | w] (DVE broadcast mult + ACT exp straight
    into the w column); num/den accumulate via matmul(O_g, Y'_g) in PSUM;
    out = num/(den+1e-16) + bias.
  * next layer's XL/XR rows are produced in the same block pass (PE
    transpose + ONE 256-col matmul against [Wl|Wr]); the XL AllGather chunks
    fire shortly after their producing block ranges complete.
"""

import os
from contextlib import ExitStack

import numpy as np
import ml_dtypes

import concourse.bacc as bacc
import concourse.mybir as mybir
import concourse.tile as tile
from concourse._compat import cdiv
from concourse.masks import make_identity
from concourse.bass_utils import run_bass_kernel_spmd

F32 = mybir.dt.float32
BF16 = mybir.dt.bfloat16
I16 = mybir.dt.int16
AX = mybir.AxisListType
OP = mybir.AluOpType
ACTF = mybir.ActivationFunctionType

D = 128
P = 128
NQ = 4          # SWDGE queues
AGC = 4         # AllGather chunks (== src buckets)
CQS = [4095, 4095, 3310, 1000]  # local rows per AllGather chunk


class Cfg:
    def __init__(self, N, cores, b1k):
        assert N % cores == 0
        self.N, self.CORES = N, cores
        self.NPC = N // cores
        assert sum(CQS) == self.NPC
        self.NBLK = cdiv(self.NPC, P)
        self.LASTW = self.NPC - (self.NBLK - 1) * P
        # bucket == AllGather chunk (chunk-major XLf layout): chunk j holds
        # rows {core c, local q in [cum[j],cum[j+1])} at
        # 8*cum[j] + c*CQS[j] + (q-cum[j])
        self.NBUCK = AGC
        self.CUM = [0]
        for q in CQS:
            self.CUM.append(self.CUM[-1] + q)
        for q in CQS:
            assert q * cores - 1 <= 32767
        self.B1K = list(b1k)
        for v in self.B1K:
            assert v % P == 0
        self.S1K = [v // P for v in self.B1K]      # groups per bucket
        self.S = sum(self.S1K)                     # groups per block
        self.S1OFF = [0]
        for v in self.S1K:
            self.S1OFF.append(self.S1OFF[-1] + v)
        # xlg pair layout: bucket k occupies cols [XG[k], XG[k]+2*B1K[k])
        self.XG = [0]
        for v in self.B1K:
            self.XG.append(self.XG[-1] + 2 * v)
        self.XLGW = self.XG[-1]
        self.NQUAD = cdiv(self.NBLK, 2)
        # idx cols per (pair,bucket): 2*B1K[k] idx wrapped in 16
        self.GIK = [2 * v // 16 for v in self.B1K]
        self.GOFF = [0]
        for v in self.GIK:
            self.GOFF.append(self.GOFF[-1] + v)
        self.GI = self.GOFF[-1]                    # idx cols per quad
        self.IDXCOLS = self.NQUAD * self.GI


def _wrap16(v):
    L = v.size
    assert L % 16 == 0
    w = v.reshape(L // 16, 16).T.astype(np.int16)
    return np.tile(w, (8, 1))


def edge_meta(cfg, edge_index):
    """Per-core (src_row, dst_local, bucket) with bucket = AG chunk."""
    src = np.asarray(edge_index[0], dtype=np.int64)
    dst = np.asarray(edge_index[1], dtype=np.int64)
    s_c, s_q = src // cfg.NPC, src % cfg.NPC
    cum = np.asarray(cfg.CUM)
    src_buck = np.searchsorted(cum, s_q, side="right") - 1
    cqs = np.asarray(CQS)
    src_row = s_c * cqs[src_buck] + (s_q - cum[src_buck])
    return src, dst, src_buck, src_row


def host_prep(cfg, edge_index):
    src, dst, src_buck, src_row = edge_meta(cfg, edge_index)
    cores = []
    for c in range(cfg.CORES):
        base = c * cfg.NPC
        m = (dst >= base) & (dst < base + cfg.NPC)
        es, ed, buck = src_row[m], dst[m] - base, src_buck[m]
        blk = ed // P
        order = np.lexsort((es, buck, blk))
        es, ed, blk, buck = es[order], ed[order], blk[order], buck[order]
        key = blk * cfg.NBUCK + buck
        bounds = np.searchsorted(key, np.arange(cfg.NBLK * cfg.NBUCK + 1))
        cnt = np.diff(bounds).reshape(cfg.NBLK, cfg.NBUCK)
        for k in range(cfg.NBUCK):
            if cnt[:, k].max() > cfg.B1K[k]:
                raise ValueError(
                    f"bucket {k} overflow: {cnt[:, k].max()} > {cfg.B1K[k]}"
                )
        idx_slots = [
            np.zeros((cfg.NBLK, cfg.B1K[k]), np.int64) for k in range(cfg.NBUCK)
        ]
        dl_slots = [
            np.full((cfg.NBLK, cfg.B1K[k]), 512.0, np.float32)
            for k in range(cfg.NBUCK)
        ]
        for b in range(cfg.NBLK):
            for k in range(cfg.NBUCK):
                i0, i1 = bounds[b * cfg.NBUCK + k], bounds[b * cfg.NBUCK + k + 1]
                n = i1 - i0
                idx_slots[k][b, :n] = es[i0:i1]
                dl_slots[k][b, :n] = (ed[i0:i1] - b * P).astype(np.float32)
        # one gather per (pair, bucket): blocks 2p,2p+1 concatenated
        cols = []
        for q_ in range(cfg.NQUAD):
            for k in range(cfg.NBUCK):
                segs = []
                for bq in range(2):
                    b0 = 2 * q_ + bq
                    if b0 < cfg.NBLK:
                        segs.append(idx_slots[k][b0])
                    else:
                        segs.append(np.zeros(cfg.B1K[k], np.int64))
                cols.append(_wrap16(np.concatenate(segs)))
        xl_idx = np.concatenate(cols, axis=1)
        # dloc per block, column order (k, t, c): slot group s = S1OFF[k]+t
        dl = np.concatenate(
            [
                dl_slots[k].reshape(cfg.NBLK, cfg.S1K[k], P)
                for k in range(cfg.NBUCK)
            ],
            axis=1,
        )  # [NBLK, S, P]
        dl = dl.transpose(2, 0, 1).reshape(P, cfg.NBLK * cfg.S)
        dloc = np.ascontiguousarray(dl).astype(ml_dtypes.bfloat16)
        cores.append(dict(xl_idx=xl_idx, dloc=dloc))
    return cores


def host_consts(cfg, Wl, Wr, att, b, x):
    Wl = np.asarray(Wl, np.float32)
    Wr = np.asarray(Wr, np.float32)
    att = np.asarray(att, np.float32)
    b = np.asarray(b, np.float32)
    x = np.asarray(x, np.float32)
    # [Wl | Wr] concatenated along the output dim: one 256-col matmul
    wlr = np.concatenate([Wl, Wr], axis=2)  # [3, 128, 256]
    wlr_all = wlr.reshape(3 * D, 2 * D).astype(ml_dtypes.bfloat16)
    att_mat = np.concatenate(
        [np.tile(att[l][None, :], (P, 1)) for l in range(3)], 0
    ).astype(ml_dtypes.bfloat16)
    bias_mat = np.concatenate(
        [np.tile(b[l][None, :], (P, 1)) for l in range(3)], 0
    ).astype(np.float32)
    # iota repeated: col value c%128, for O build [P, S*128]
    iota = np.tile(np.arange(P, dtype=np.float32)[None, :], (P, cfg.S)).astype(
        ml_dtypes.bfloat16
    )
    out = []
    for c in range(cfg.CORES):
        xT = np.ascontiguousarray(x[c * cfg.NPC : (c + 1) * cfg.NPC].T).astype(
            ml_dtypes.bfloat16
        )
        out.append(
            dict(
                xT_loc=xT,
                Wlr_all=wlr_all,
                att_mat=att_mat,
                bias_mat=bias_mat,
                iota_mat=iota,
            )
        )
    return out


def build_program(cfg):
    nc = bacc.Bacc(
        "TRN2", target_bir_lowering=False, debug=False,
        num_devices=cfg.CORES, num_swdge_queues=NQ,
    )
    NPC, NBLK, NBUCK, S = cfg.NPC, cfg.NBLK, cfg.NBUCK, cfg.S
    S1K, S1OFF, B1K, XG = cfg.S1K, cfg.S1OFF, cfg.B1K, cfg.XG
    GI, GOFF = cfg.GI, cfg.GOFF

    # group s -> (bucket k, t within bucket)
    def s2kt(s):
        for k in range(NBUCK):
            if s < S1OFF[k + 1]:
                return k, s - S1OFF[k]
        raise AssertionError

    # bucket-aligned chunks of <=4 groups: each chunk is ONE contiguous run
    # in xlg, so the identity matmul's start=True covers the whole region it
    # initializes (multiple start writes into one PSUM bank lose data, and
    # strict sequencing of runs stalls the PE queue).
    VCH = []
    for k in range(NBUCK):
        t = 0
        while t < S1K[k]:
            vn = min(4, S1K[k] - t)
            VCH.append((S1OFF[k] + t, vn, [(k, t, vn)]))
            t += vn

    xT_loc = nc.dram_tensor("xT_loc", [P, NPC], BF16, kind="ExternalInput")
    Wlr_all = nc.dram_tensor("Wlr_all", [3 * D, 2 * D], BF16, kind="ExternalInput")
    att_mat = nc.dram_tensor("att_mat", [3 * P, D], BF16, kind="ExternalInput")
    bias_mat = nc.dram_tensor("bias_mat", [3 * P, D], F32, kind="ExternalInput")
    iota_mat = nc.dram_tensor("iota_mat", [P, S * P], BF16, kind="ExternalInput")
    xl_idx = nc.dram_tensor("xl_idx", [P, cfg.IDXCOLS], I16, kind="ExternalInput")
    dloc_in = nc.dram_tensor("dloc", [P, NBLK * S], BF16, kind="ExternalInput")
    out_loc = nc.dram_tensor("out_loc", [NPC, D], F32, kind="ExternalOutput")

    XLb = [nc.dram_tensor(f"XLb{l}", [NPC, D], BF16) for l in range(3)]
    XR = [nc.dram_tensor(f"XR{l}", [NPC, D], BF16) for l in range(3)]
    XLf = [
        nc.dram_tensor(f"XLf{l}", [cfg.N, D], BF16, addr_space="Shared")
        for l in range(3)
    ]

    # AllGather chunk j gathers local rows [CUM[j],CUM[j+1]) of every core
    # into the chunk-major slice [8*CUM[j], 8*CUM[j+1]) of XLf.
    CUM = cfg.CUM
    AG_DELAY = 4
    cb = [min(cdiv(CUM[j + 1], P) + AG_DELAY, NBLK) for j in range(AGC)]

    def ag_chunk(l, j):
        ins = XLb[l][CUM[j] : CUM[j + 1], :].opt()
        outs = XLf[l][8 * CUM[j] : 8 * CUM[j + 1], :].opt()
        nc.gpsimd.collective_compute(
            "AllGather", OP.bypass,
            replica_groups=[list(range(cfg.CORES))],
            ins=[ins], outs=[outs],
        )

    with tile.TileContext(nc) as tc, ExitStack() as ctx:
        consts = ctx.enter_context(tc.tile_pool(name="consts", bufs=1))
        gpool = ctx.enter_context(tc.tile_pool(name="gath", bufs=5))
        opool = ctx.enter_context(tc.tile_pool(name="opool", bufs=5))
        otp = ctx.enter_context(tc.tile_pool(name="otp", bufs=6))
        wrk = ctx.enter_context(tc.tile_pool(name="wrk", bufs=4))
        small = ctx.enter_context(tc.tile_pool(name="small", bufs=5))
        xrp = ctx.enter_context(tc.tile_pool(name="xrp", bufs=4))
        psV = ctx.enter_context(tc.tile_pool(name="psV", bufs=2, space="PSUM"))
        psO = ctx.enter_context(tc.tile_pool(name="psO", bufs=2, space="PSUM"))
        psA = ctx.enter_context(tc.tile_pool(name="psA", bufs=2, space="PSUM"))
        psF = ctx.enter_context(tc.tile_pool(name="psF", bufs=1, space="PSUM"))

        # long-lived ping-pong PSUM banks for XL/XR production (even/odd
        # blocks use disjoint halves -> depth-2 pipeline from one bank each)
        pfin = psF.tile([P, 4 * D], F32, tag="fin")
        pfint = psF.tile([P, 2 * P], BF16, tag="fint")

        iota_t = consts.tile([P, S * P], BF16, tag="iota")
        nc.sync.dma_start(iota_t[:], iota_mat[:, :])
        dlt = consts.tile([P, NBLK * S], BF16, tag="dlt")
        nc.sync.dma_start(dlt[:], dloc_in[:, :])
        idxt = consts.tile([P, cfg.IDXCOLS], I16, tag="idx")
        nc.sync.dma_start(idxt[:], xl_idx[:, :])
        ident_t = consts.tile([P, P], BF16, tag="ident")
        make_identity(nc, ident_t[:])
        ones_t = consts.tile([P, 1], BF16, tag="ones")
        nc.vector.memset(ones_t[:], 1.0)
        wlr_t, at_t, bi_t = [], [], []
        for l in range(3):
            w1 = consts.tile([P, 2 * D], BF16, tag=f"wlr{l}")
            nc.sync.dma_start(w1[:], Wlr_all[l * D : (l + 1) * D, :])
            a1 = consts.tile([P, D], BF16, tag=f"att{l}")
            nc.sync.dma_start(a1[:], att_mat[l * P : (l + 1) * P, :])
            b1t = consts.tile([P, D], F32, tag=f"bias{l}")
            nc.sync.dma_start(b1t[:], bias_mat[l * P : (l + 1) * P, :])
            wlr_t.append(w1); at_t.append(a1); bi_t.append(b1t)

        # ---- layer-0 XL/XR production + chunked AllGather ----
        nag = 0
        for cblk in range(NBLK):
            cw = P if cblk < NBLK - 1 else cfg.LASTW
            if cblk % 4 == 0:
                xw = min(4 * P, NPC - cblk * P)
                xTs4 = wrk.tile([P, 4 * P], BF16, tag="xTs4")
                nc.sync.dma_start(
                    xTs4[:, :xw], xT_loc[:, cblk * P : cblk * P + xw]
                )
            xTs = xTs4[:, (cblk % 4) * P : (cblk % 4) * P + cw]
            pxlr = pfin[:, (cblk % 2) * 2 * D : ((cblk % 2) + 1) * 2 * D]
            nc.tensor.matmul(
                pxlr[:cw, :], xTs[:, :cw], wlr_t[0][:], start=True, stop=True
            )
            sxlr = small.tile([P, 2 * D], BF16, tag="sxlr")
            nc.scalar.activation(sxlr[:cw, :], pxlr[:cw, :], ACTF.Copy)
            nc.scalar.dma_start(XLb[0][cblk * P : cblk * P + cw, :], sxlr[:cw, :D])
            nc.sync.dma_start(XR[0][cblk * P : cblk * P + cw, :], sxlr[:cw, D:])
            if nag < AGC and cblk == cb[nag] - 1:
                ag_chunk(0, nag)
                nag += 1

        for l in range(3):
            nag = 0
            for b in range(NBLK):
                bw = P if b < NBLK - 1 else cfg.LASTW
                pr, bb = divmod(b, 2)
                # -- gathers for this block's pair (issued once per pair) --
                if bb == 0:
                    xlg = gpool.tile([P, cfg.XLGW], BF16, tag="xlg")
                    for k in range(NBUCK):
                        kb = 8 * CUM[k]
                        ke = 8 * CUM[k + 1]
                        ic0 = pr * GI + GOFF[k]
                        nc.gpsimd.dma_gather(
                            xlg[:, XG[k] : XG[k] + 2 * B1K[k]].rearrange(
                                "p (m x) -> p m x", x=D
                            ),
                            XLf[l][kb:ke, :],
                            idxt[:, ic0 : ic0 + cfg.GIK[k]],
                            2 * B1K[k], 2 * B1K[k], D,
                            single_packet=False,
                            queue_num=(pr * 4 + k) % NQ,
                        )

                # -- XR block (local rows) --
                xrb = xrp.tile([P, D], BF16, tag="xrb")
                if bw < P:
                    nc.vector.memset(xrb[:], 0.0)
                nc.sync.dma_start(xrb[:bw, :], XR[l][b * P : b * P + bw, :])

                # -- O build: one DVE op for the whole block --
                O = opool.tile([P, S * P], BF16, tag="O")
                nc.vector.tensor_tensor(
                    O[:].rearrange("p (s c) -> p s c", c=P),
                    iota_t[:].rearrange("p (s c) -> p s c", c=P),
                    dlt[:, b * S : (b + 1) * S].unsqueeze(2).to_broadcast(
                        [P, S, P]
                    ),
                    op=OP.is_equal,
                )

                # -- v = O_T_g.T @ XR_blk + xl (wide ident runs) in PSUM --
                # transposes/sot copies batch in flat 4-group chunks (each
                # transpose is its own start+stop matmul, so no PSUM
                # multi-start hazard); pv chunks stay bucket-aligned.
                z = wrk.tile([P, S * P], BF16, tag="z")
                sots = []
                for f0 in range(0, S, 4):
                    fn = min(4, S - f0)
                    pot = psO.tile([P, 4 * P], BF16, tag="ot")
                    sot = otp.tile([P, 4 * P], BF16, tag="sot")
                    for gi in range(fn):
                        g = f0 + gi
                        nc.tensor.transpose(
                            pot[:, gi * P : (gi + 1) * P],
                            O[:, g * P : (g + 1) * P], ident_t[:],
                        )
                    nc.scalar.activation(
                        sot[:, : fn * P], pot[:, : fn * P], ACTF.Copy
                    )
                    sots.append(sot)
                for (v0, vn, runs) in VCH:
                    pv = psV.tile([P, 4 * P], F32, tag="v")
                    for (k, t, rl) in runs:
                        off = (S1OFF[k] + t) - v0
                        nc.tensor.matmul(
                            pv[:, off * P : (off + rl) * P],
                            ident_t[:],
                            xlg[
                                :,
                                XG[k] + (bb * S1K[k] + t) * P : XG[k]
                                + (bb * S1K[k] + t + rl) * P,
                            ],
                            start=True, stop=False,
                        )
                    for gi in range(vn):
                        g = v0 + gi
                        nc.tensor.matmul(
                            pv[:, gi * P : (gi + 1) * P],
                            sots[g // 4][:, (g % 4) * P : (g % 4 + 1) * P],
                            xrb[:],
                            start=False, stop=True,
                            skip_group_check=True,
                        )
                    nc.scalar.activation(
                        z[:, v0 * P : (v0 + vn) * P], pv[:, : vn * P],
                        ACTF.Prelu, alpha=0.2,
                    )

                # -- scores: t = z*att; e = sum_d t (halve + reduce) --
                t = wrk.tile([P, S * P], BF16, tag="t")
                nc.vector.tensor_tensor(
                    t[:].rearrange("p (s x) -> p s x", x=D),
                    z[:].rearrange("p (s x) -> p s x", x=D),
                    at_t[l][:].unsqueeze(1).to_broadcast([P, S, D]),
                    op=OP.mult,
                )
                h = wrk.tile([P, S * 64], BF16, tag="h")
                t3 = t[:].rearrange("p (s x) -> p s x", x=D)
                nc.vector.tensor_tensor(
                    h[:].rearrange("p (s x) -> p s x", x=64),
                    t3[:, :, :64], t3[:, :, 64:],
                    op=OP.add,
                )
                sc = small.tile([P, S], F32, tag="sc")
                nc.vector.tensor_reduce(
                    sc[:], h[:].rearrange("p (s x) -> p s x", x=64),
                    axis=AX.X, op=OP.add,
                )
                # -- Y' = [w*xl | w | pad], num/den matmuls --
                # exp(e) goes straight into the w column; the Yp build then
                # reads that column back as its broadcast multiplier.
                Yp = wrk.tile([P, S * 132], BF16, tag="Yp")
                Yp3 = Yp[:].rearrange("p (s c) -> p s c", c=132)
                nc.scalar.activation(
                    Yp3[:, :, D : D + 1], sc[:].unsqueeze(2), ACTF.Exp,
                )
                for k in range(NBUCK):
                    nc.vector.tensor_tensor(
                        Yp3[:, S1OFF[k] : S1OFF[k + 1], :D],
                        xlg[
                            :, XG[k] + bb * B1K[k] : XG[k] + (bb + 1) * B1K[k]
                        ].rearrange("p (t c) -> p t c", c=P),
                        Yp3[:, S1OFF[k] : S1OFF[k + 1], D : D + 1].to_broadcast(
                            [P, S1K[k], D]
                        ),
                        op=OP.mult,
                    )
                pa = psA.tile([P, 132], F32, tag="pa")
                for g in range(S):
                    nc.tensor.matmul(
                        pa[:], O[:, g * P : (g + 1) * P],
                        Yp[:, g * 132 : (g + 1) * 132],
                        start=(g == 0), stop=(g == S - 1),
                    )

                # -- output stage --
                den = small.tile([P, 1], F32, tag="den")
                nc.vector.tensor_scalar(
                    den[:], pa[:, D : D + 1], 1e-16, None, op0=OP.add
                )
                rec = small.tile([P, 1], F32, tag="rec")
                nc.vector.reciprocal(rec[:], den[:])
                onum = small.tile([P, D], F32, tag="onum")
                nc.vector.tensor_scalar(
                    onum[:], pa[:, :D], rec[:], None, op0=OP.mult
                )
                nc.vector.tensor_tensor(onum[:], onum[:], bi_t[l][:], op=OP.add)
                if l == 2:
                    nc.sync.dma_start(out_loc[b * P : b * P + bw, :], onum[:bw, :])
                else:
                    hrow = small.tile([P, D], BF16, tag="hrow")
                    nc.scalar.activation(hrow[:], onum[:], ACTF.Relu)
                    pst = pfint[:, (b % 2) * P : ((b % 2) + 1) * P]
                    nc.tensor.transpose(pst[:], hrow[:], ident_t[:])
                    hT = small.tile([P, P], BF16, tag="hT")
                    nc.scalar.activation(hT[:], pst[:], ACTF.Copy)
                    pxlr = pfin[:, (b % 2) * 2 * D : ((b % 2) + 1) * 2 * D]
                    nc.tensor.matmul(
                        pxlr[:], hT[:], wlr_t[l + 1][:], start=True, stop=True
                    )
                    sxlr = small.tile([P, 2 * D], BF16, tag="sxlr")
                    nc.scalar.activation(sxlr[:], pxlr[:], ACTF.Copy)
                    nc.sync.dma_start(
                        XLb[l + 1][b * P : b * P + bw, :], sxlr[:bw, :D]
                    )
                    nc.sync.dma_start(
                        XR[l + 1][b * P : b * P + bw, :], sxlr[:bw, D:]
                    )
                    if nag < AGC and b == cb[nag] - 1:
                        ag_chunk(l + 1, nag)
                        nag += 1
    nc.compile()
    return nc


def kernel(x, Wl, Wr, att, b, edge_index):
    x = np.asarray(x, np.float32)
    edge_index = np.asarray(edge_index)
    N = x.shape[0]
    CORES = 8

    # per-bucket slot budgets: max count over (core, block) per bucket
    npc = N // CORES
    nblk = cdiv(npc, P)
    cum = [0]
    for q in CQS:
        cum.append(cum[-1] + q)
    src = np.asarray(edge_index[0], np.int64)
    dst = np.asarray(edge_index[1], np.int64)
    s_q = src % npc
    sbuck = np.searchsorted(np.asarray(cum), s_q, side="right") - 1
    b1k = []
    for k in range(AGC):
        mx = 0
        for c in range(CORES):
            m = (dst >= c * npc) & (dst < (c + 1) * npc) & (sbuck == k)
            if m.any():
                key = (dst[m] - c * npc) // P
                mx = max(mx, int(np.bincount(key, minlength=nblk).max()))
        b1k.append(max(cdiv(mx, P) * P, P))

    cfg = Cfg(N=N, cores=CORES, b1k=b1k)
    idx_data = host_prep(cfg, edge_index)
    const_data = host_consts(cfg, Wl, Wr, att, b, x)
    nc = build_program(cfg)
    in_maps = [{**idx_data[c], **const_data[c]} for c in range(CORES)]

    prof_dir = os.environ.get("GAT_PROFILE", "")
    if prof_dir:
        import sys
        sys.path.insert(0, "/root/.axon_site")
        from trn_agent_boot import trn_boot
        hook = trn_boot._ntff_profile_via_ctypes("/opt/axon/libaxon_pjrt.so")
        os.makedirs(prof_dir, exist_ok=True)
        with hook(prof_dir, [0]):
            res = run_bass_kernel_spmd(nc, in_maps, core_ids=list(range(CORES)))
    else:
        res = run_bass_kernel_spmd(nc, in_maps, core_ids=list(range(CORES)))

    out = np.concatenate([r["out_loc"] for r in res.results], axis=0)
    return out.astype(np.float32)
